# revision 1
# baseline (speedup 1.0000x reference)
"""Graph Wavelet NN (2-layer) Trainium2 kernel, 8-core row-parallel.

Math per layer:  out = (wavelets * filt[None,:]) @ (wavelets_inv @ (x @ W))
Sharding: core i owns row-block rows_i (1024 rows).
  - s = wavelets_inv @ t is computed as sum over cores of
    Winv[:, cols_i] @ t[rows_i]  -> AllReduce (one 8MB collective per layer).
  - out rows: (wavelets*filt)[rows_i, :] @ s  (local after AllReduce).
Host pre-transposes the stationary operands:
  winvT = wavelets_inv.T[rows_i, :]          [1024, 8192]
  a1    = (wavelets[rows_i]*f1).T            [8192, 1024]
  a2    = (wavelets[rows_i]*f2).T            [8192, 1024]
  xT    = input.T[:, rows_i]                 [512, 1024]
Big streams (winvT, a1, a2, AR payload) in bf16, small matmuls in float32r;
all accumulation fp32 in PSUM. Verified on HW: rel err 6.3e-3.
"""

import sys

if "/opt/trn_rl_repo" not in sys.path:
    sys.path.insert(0, "/opt/trn_rl_repo")

import numpy as np

import concourse.bass as bass
import concourse.mybir as mybir
import concourse.tile as tile
from concourse import bacc, bass_utils

N = 8192
F = 512
C = 256
NCORES = 8
R = N // NCORES  # 1024 rows per core

F32 = mybir.dt.float32
F32R = mybir.dt.float32r
BF16 = mybir.dt.bfloat16
USE_BF16 = True
BIG = BF16 if USE_BF16 else F32R
import ml_dtypes
NP_BIG = ml_dtypes.bfloat16 if USE_BF16 else np.float32

MBLK = 512                  # P-phase output row block
NMBLK = N // MBLK           # 16
NKC_LOC = R // 128          # 8   local-k chunks (P phases)
NKC_F = F // 128            # 4   k chunks for x @ W1
NKC_GLOB = N // 128         # 64  global-k chunks (h/out phases)
NMT = R // 128              # 8   local row tiles


def r(ap):
    return ap.bitcast(F32R)


def build_kernel(sim_single_core=False):
    nc = bacc.Bacc(
        "TRN2",
        target_bir_lowering=False,
        debug=False,
        num_devices=1 if sim_single_core else NCORES,
    )

    xT = nc.dram_tensor("xT", [F, R], F32, kind="ExternalInput")
    w1 = nc.dram_tensor("w1", [F, C], F32, kind="ExternalInput")
    w2 = nc.dram_tensor("w2", [C, C], F32, kind="ExternalInput")
    winvT = nc.dram_tensor("winvT", [R, N], BIG if USE_BF16 else F32, kind="ExternalInput")
    a1 = nc.dram_tensor("a1", [N, R], BIG if USE_BF16 else F32, kind="ExternalInput")
    a2 = nc.dram_tensor("a2", [N, R], BIG if USE_BF16 else F32, kind="ExternalInput")
    outT = nc.dram_tensor("outT", [C, R], F32, kind="ExternalOutput")

    rg = [list(range(NCORES))]

    with tile.TileContext(nc) as tc:
        with (
            tc.tile_pool(name="dram", bufs=1, space="DRAM") as dram,
            tc.tile_pool(name="const", bufs=1) as const,
            tc.tile_pool(name="winvp", bufs=6) as winvp,
            tc.tile_pool(name="ap_pool", bufs=6) as ap_pool,
            tc.tile_pool(name="stage", bufs=4) as stage,
            tc.tile_pool(name="psum256", bufs=4, space="PSUM") as psum256,
            tc.tile_pool(name="psum1k", bufs=2, space="PSUM") as psum1k,
        ):
            p1_d = dram.tile([N, C], BIG)
            p2_d = dram.tile([N, C], BIG)
            s1_d = dram.tile([N, C], BIG, addr_space="Shared")
            s2_d = dram.tile([N, C], BIG, addr_space="Shared")

            # ---- persistent SBUF ----
            xT_sb = const.tile([128, NKC_F, R], F32R)      # 16KB/part
            w1_sb = const.tile([128, NKC_F, C], F32R)      # 4KB/part
            w2_sb = const.tile([128, 2, C], F32R)          # 2KB/part
            t1_sb = const.tile([128, NKC_LOC, C], BIG)    # 8KB/part
            t2_sb = const.tile([128, NKC_LOC, C], BIG)    # 8KB/part
            s_sb = const.tile([128, NKC_GLOB, C], BIG)    # 64KB/part
            h1T_sb = const.tile([128, 2, R], F32R)         # 8KB/part

            nc.sync.dma_start(
                out=xT_sb[:], in_=xT.ap().rearrange("(kc p) m -> p kc m", p=128).bitcast(F32R)
            )
            nc.sync.dma_start(
                out=w1_sb[:], in_=w1.ap().rearrange("(kc p) n -> p kc n", p=128).bitcast(F32R)
            )
            nc.sync.dma_start(
                out=w2_sb[:], in_=w2.ap().rearrange("(kc p) n -> p kc n", p=128).bitcast(F32R)
            )

            # ---- phase 0: t1 = x @ W1  (local rows) ----
            for mt in range(NMT):
                pt = psum256.tile([128, C], F32, tag="acc256")
                for kc in range(NKC_F):
                    nc.tensor.matmul(
                        pt[:],
                        xT_sb[:, kc, mt * 128:(mt + 1) * 128],
                        w1_sb[:, kc, :],
                        start=(kc == 0),
                        stop=(kc == NKC_F - 1),
                    )
                nc.vector.tensor_copy(t1_sb[:, mt, :], pt[:])

            # ---- P phases: partial = Winv[:, cols_i] @ t ----
            def p_phase(t_sb, p_d):
                for mb in range(NMBLK):
                    wv = winvp.tile([128, NKC_LOC, MBLK], BIG, tag="wv")
                    nc.sync.dma_start(
                        out=wv[:],
                        in_=winvT.ap()[:, mb * MBLK:(mb + 1) * MBLK].rearrange(
                            "(kc p) m -> p kc m", p=128
                        ),
                    )
                    st = stage.tile([128, MBLK // 128, C], BIG, tag="st4")
                    for mt in range(MBLK // 128):
                        pt = psum256.tile([128, C], F32, tag="acc256")
                        for kc in range(NKC_LOC):
                            nc.tensor.matmul(
                                pt[:],
                                wv[:, kc, mt * 128:(mt + 1) * 128],
                                t_sb[:, kc, :],
                                start=(kc == 0),
                                stop=(kc == NKC_LOC - 1),
                            )
                        nc.vector.tensor_copy(st[:, mt, :], pt[:])
                    row0 = mb * MBLK
                    nc.sync.dma_start(
                        out=p_d[row0:row0 + MBLK, :].rearrange(
                            "(mt p) n -> p mt n", p=128
                        ),
                        in_=st[:],
                    )

            # ---- big phases: accT = A.T @ s  ([C, R] in psum) ----
            def big_phase(a_dram, out_cb):
                accs = [
                    psum1k.tile([128, R], F32, tag="acc1k", name=f"acc1k_{j}")
                    for j in range(2)
                ]
                for kci in range(NKC_GLOB // 2):
                    at = ap_pool.tile([128, 2, R], BIG, tag="at")
                    for sub in range(2):
                        kc = kci * 2 + sub
                        nc.sync.dma_start(
                            out=at[:, sub, :],
                            in_=a_dram.ap()[kc * 128:(kc + 1) * 128, :],
                        )
                    for sub in range(2):
                        kc = kci * 2 + sub
                        for nch in range(2):
                            for mh in range(2):
                                nc.tensor.matmul(
                                    accs[nch][:, mh * 512:(mh + 1) * 512],
                                    s_sb[:, kc, nch * 128:(nch + 1) * 128],
                                    at[:, sub, mh * 512:(mh + 1) * 512],
                                    start=(kc == 0),
                                    stop=(kc == NKC_GLOB - 1),
                                )
                for nch in range(2):
                    out_cb(nch, accs[nch])

            def load_s(s_d):
                for q in range(4):
                    kc0 = q * (NKC_GLOB // 4)
                    nc.sync.dma_start(
                        out=s_sb[:, kc0:kc0 + NKC_GLOB // 4, :],
                        in_=s_d[kc0 * 128:(kc0 + NKC_GLOB // 4) * 128, :].rearrange(
                            "(kc p) n -> p kc n", p=128
                        ),
                    )

            def all_reduce(p_d, s_d):
                if sim_single_core:
                    nc.sync.dma_start(out=s_d[:, :], in_=p_d[:, :])
                else:
                    nc.gpsimd.collective_compute(
                        "AllReduce",
                        mybir.AluOpType.add,
                        replica_groups=rg,
                        ins=[p_d.opt()],
                        outs=[s_d.opt()],
                    )

            # ================= layer 1 =================
            p_phase(t1_sb, p1_d)
            all_reduce(p1_d, s1_d)
            load_s(s1_d)

            def relu_out(nch, acc):
                nc.scalar.activation(
                    h1T_sb[:, nch, :], acc[:], mybir.ActivationFunctionType.Relu
                )

            big_phase(a1, relu_out)

            # t2 = h1 @ W2
            for mt in range(NMT):
                pt = psum256.tile([128, C], F32, tag="acc256")
                for kc in range(2):
                    nc.tensor.matmul(
                        pt[:],
                        h1T_sb[:, kc, mt * 128:(mt + 1) * 128],
                        w2_sb[:, kc, :],
                        start=(kc == 0),
                        stop=(kc == 1),
                    )
                nc.vector.tensor_copy(t2_sb[:, mt, :], pt[:])

            # ================= layer 2 =================
            p_phase(t2_sb, p2_d)
            all_reduce(p2_d, s2_d)
            load_s(s2_d)

            def store_out(nch, acc):
                # h1T_sb is dead after the t2 phase; reuse it as staging
                nc.vector.tensor_copy(h1T_sb[:, nch, :], acc[:])
                nc.sync.dma_start(
                    out=outT.ap()[nch * 128:(nch + 1) * 128, :].bitcast(F32R),
                    in_=h1T_sb[:, nch, :],
                )

            big_phase(a2, store_out)

    nc.compile()
    return nc


_NC_CACHE = {}


def _get_nc():
    if "nc" not in _NC_CACHE:
        _NC_CACHE["nc"] = build_kernel()
    return _NC_CACHE["nc"]


def make_in_maps(input, wavelets, wavelets_inv, W1, W2, filter1, filter2):
    input = np.asarray(input, np.float32)
    wavelets = np.asarray(wavelets, np.float32)
    wavelets_inv = np.asarray(wavelets_inv, np.float32)
    W1 = np.ascontiguousarray(np.asarray(W1, np.float32))
    W2 = np.ascontiguousarray(np.asarray(W2, np.float32))
    filter1 = np.asarray(filter1, np.float32)
    filter2 = np.asarray(filter2, np.float32)

    xTf = np.ascontiguousarray(input.T)          # [F, N]
    in_maps = []
    for i in range(NCORES):
        r0, r1 = i * R, (i + 1) * R
        in_maps.append(
            {
                "xT": np.ascontiguousarray(xTf[:, r0:r1]),
                "w1": W1,
                "w2": W2,
                "winvT": np.ascontiguousarray(wavelets_inv[:, r0:r1].T).astype(NP_BIG),
                "a1": np.ascontiguousarray((wavelets[r0:r1] * filter1).T).astype(NP_BIG),
                "a2": np.ascontiguousarray((wavelets[r0:r1] * filter2).T).astype(NP_BIG),
            }
        )
    return in_maps


def run(in_maps, trace=False, **kw):
    nc = _get_nc()
    return bass_utils.run_bass_kernel_spmd(
        nc, in_maps, core_ids=list(range(NCORES)), trace=trace, **kw
    )


def kernel(input, wavelets, wavelets_inv, W1, W2, filter1, filter2):
    in_maps = make_in_maps(
        input, wavelets, wavelets_inv, W1, W2, filter1, filter2
    )
    res = run(in_maps)
    out = np.empty((N, C), np.float32)
    for i in range(NCORES):
        out[i * R:(i + 1) * R, :] = res.results[i]["outT"].T
    return out



# revision 2
# speedup vs baseline: 1.1374x; 1.1374x over previous
"""GWNN 2-layer Trainium2 kernel v2 — 8-core row-parallel, AllGather-based.

Differences vs v1 (AllReduce-based):
  - Row-shard BOTH wavelets and wavelets_inv: core i computes s rows Ui and
    out rows Ri directly; cross-core exchange is 4 chunked AllGathers of the
    small [N, C] activations (t1, s1, t2, s2) instead of 2 AllReduces.
    AG moves ~half the bytes of AR and overlaps with matmuls.
  - Filters are applied to s on device (diag(f) @ winv @ t == row-scale of s),
    so layer 1 and layer 2 consume the SAME wavelets row-block: it is loaded
    once and cached in SBUF (16 MB) across both layers. winvT streams twice.
  - All gathers are 2 chunks (128 + 896 local rows) so the consumer GEMM can
    start ~6 us after the producer finishes instead of waiting a full AG.
  - Contraction (gathered) dim is permuted host-side (pi) to match AG chunk
    order: chunk-major, then rank-major, then row.

Math per layer: out = (wavelets * f[None,:]) @ (wavelets_inv @ (x @ W))
             == wavelets @ (f[:,None] * (wavelets_inv @ (x @ W)))

build_kernel(reps=K) repeats the whole body K times (same inputs/outputs) so
host-side wall timing of reps=a vs reps=b isolates the device body time from
the ~2ms axon dispatch floor.
"""

import sys

if "/opt/trn_rl_repo" not in sys.path:
    sys.path.insert(0, "/opt/trn_rl_repo")

import numpy as np
import ml_dtypes

import concourse.bass as bass
import concourse.mybir as mybir
import concourse.tile as tile
from concourse import bacc, bass_utils

N = 8192
F = 512
C = 256
NCORES = 8
R = N // NCORES          # 1024 rows per core
NKC = N // 128           # 64 contraction chunks
NMT = R // 128           # 8 local row tiles
CHUNKS = [(0, 128), (128, 256), (384, 640)]  # default AG ladder (off, nrows)

F32 = mybir.dt.float32
BF16 = mybir.dt.bfloat16
NP_BF16 = ml_dtypes.bfloat16


def build_kernel(reps=1, upto=6, nocc=False, g1mode='norm', chunks=None):
    chunks = list(chunks) if chunks is not None else CHUNKS
    nc = bacc.Bacc("TRN2", target_bir_lowering=False, debug=False,
                   num_devices=NCORES)

    xT = nc.dram_tensor("xT", [128, F // 128, R], BF16, kind="ExternalInput")
    w1 = nc.dram_tensor("w1", [128, F // 128, C], BF16, kind="ExternalInput")
    w2 = nc.dram_tensor("w2", [128, C // 128, C], BF16, kind="ExternalInput")
    winvT = nc.dram_tensor("winvT", [NKC // 2, 128, 2, R], BF16,
                           kind="ExternalInput")
    waveT = nc.dram_tensor("waveT", [NKC // 4, 128, 4, R], BF16,
                           kind="ExternalInput")
    f1 = nc.dram_tensor("f1", [128, NMT], F32, kind="ExternalInput")
    f2 = nc.dram_tensor("f2", [128, NMT], F32, kind="ExternalInput")
    outT = nc.dram_tensor("outT", [C, R], F32, kind="ExternalOutput")

    rg = [list(range(NCORES))]
    WPRE = 5          # winv pairs pre-issued before each winv phase
    WBUFS = WPRE + 1

    with tile.TileContext(nc) as tc:
        with (
            tc.tile_pool(name="dram", bufs=1, space="DRAM") as dram,
            tc.tile_pool(name="const", bufs=1) as const,
            tc.tile_pool(name="wavep", bufs=1) as wavep,
            tc.tile_pool(name="winvp", bufs=WBUFS) as winvp,
            tc.tile_pool(name="tpcs", bufs=3) as tpcs,
            tc.tile_pool(name="spcs", bufs=3) as spcs,
            tc.tile_pool(name="stg", bufs=4) as stg,
            tc.tile_pool(name="ostg", bufs=2) as ostg,
            tc.tile_pool(name="psum", bufs=8, space="PSUM") as psum,
        ):
            # persistent SBUF (shared across reps)
            waveT_sb = wavep.tile([128, NKC, R], BF16)      # 128KB/part
            xT_sb = const.tile([128, F // 128, R], BF16)    # 8KB/part
            w1_sb = const.tile([128, F // 128, C], BF16)    # 2KB/part
            w2_sb = const.tile([128, C // 128, C], BF16)    # 1KB/part
            f1_sb = const.tile([128, NMT], F32)
            f2_sb = const.tile([128, NMT], F32)
            h1T_sb = const.tile([128, 2, R], BF16)          # 4KB/part

            for rep in range(reps):
                emit_body(nc, tc, rep, rg, WPRE, dram, winvp, tpcs, spcs,
                          stg, ostg, psum,
                          waveT_sb, xT_sb, w1_sb, w2_sb, f1_sb, f2_sb, h1T_sb,
                          xT, w1, w2, winvT, waveT, f1, f2, outT, upto, nocc, g1mode, chunks)

    nc.compile()
    return nc


def emit_body(nc, tc, rep, rg, WPRE, dram, winvp, tpcs, spcs, stg, ostg, psum,
              waveT_sb, xT_sb, w1_sb, w2_sb, f1_sb, f2_sb, h1T_sb,
              xT, w1, w2, winvT, waveT, f1, f2, outT, upto=6, nocc=False, g1mode='norm', chunks=CHUNKS):
    CH = chunks
    # ---- DRAM scratch: AG inputs (Local, p-major per tier) and outputs ----
    # tier ci holds local rows [off, off+n) stored as [128, n//128, C]
    # (partition-major) so the gathered per-rank reads are contiguous per
    # partition (128 DMA descriptors instead of 1024).
    def ag_bufs(pfx):
        locs = [dram.tile([128, n // 128, C], BF16,
                          name=f"{pfx}loc{ci}_{rep}")
                for ci, (off, n) in enumerate(CH)]
        gs = [dram.tile([NCORES * n, C], BF16, addr_space="Shared",
                        name=f"{pfx}g{ci}_{rep}")
              for ci, (off, n) in enumerate(CH)]
        return locs, gs

    t1loc, t1gs = ag_bufs("t1")
    s1loc, s1gs = ag_bufs("s1")
    t2loc, t2gs = ag_bufs("t2")
    s2loc, s2gs = ag_bufs("s2")

    nc.sync.dma_start(out=xT_sb[:], in_=xT.ap())
    nc.sync.dma_start(out=w1_sb[:], in_=w1.ap())
    nc.sync.dma_start(out=w2_sb[:], in_=w2.ap())
    nc.sync.dma_start(out=f1_sb[:], in_=f1.ap())
    nc.sync.dma_start(out=f2_sb[:], in_=f2.ap())

    def fill_wave(q, engine=None, after=None):
        # one contiguous 1MB fill -> kc 4q..4q+3; `after` chains the fill
        # behind a runtime-paced DMA so the scheduler can't hoist it into a
        # head-of-phase flood.
        inst = (engine or nc.sync).dma_start(
            out=waveT_sb[:, 4 * q:4 * q + 4, :], in_=waveT.ap()[q])
        if after is not None:
            bass._add_dep_helper(inst.ins, after.ins, True,
                                 "pace waveT fill behind stream")
        return inst

    def issue_winv_pair(j, tag):
        wv = winvp.tile([128, 2, R], BF16, tag="wv",
                        name=f"wv_{tag}_{j}_{rep}")
        inst = nc.sync.dma_start(out=wv[:], in_=winvT.ap()[j])
        return wv, inst

    def issue_piece(pool, gs, item, tag):
        # item: ("t0", None) = tier-0 piece [128, NCORES, C] (rank-major),
        #       (ci, c)      = rank c's slice of tier ci, [128, n//128, C]
        # ACT queue: piece loads wait on AG semaphores; keeping them off
        # the SP queue avoids head-of-line blocking the streaming DMAs.
        ci, c = item
        if ci == 0:
            tp = pool.tile([128, 1, C], BF16, tag="pc0",
                           name=f"pc0_{tag}_{c}_{rep}", bufs=3)
            inst = nc.scalar.dma_start(
                out=tp[:, 0, :], in_=gs[0][c * 128:(c + 1) * 128, :])
        else:
            kw = CH[ci][1] // 128
            tp = pool.tile([128, kw, C], BF16, tag=f"pc{ci}",
                           name=f"pc{ci}_{tag}_{c}_{rep}", bufs=3)
            inst = nc.scalar.dma_start(
                out=tp[:],
                in_=gs[ci][c * CH[ci][1]:(c + 1) * CH[ci][1], :].rearrange(
                    "(p kc) n -> p kc n", p=128))
        return tp, inst

    # kc (global contraction chunk) -> (piece item, index within piece)
    def kc_map():
        m = []
        pos = 0
        for ci, (off, n) in enumerate(CH):
            kw = n // 128
            if ci == 0:
                for c in range(NCORES):
                    m.append(((0, c), 0))
            else:
                for c in range(NCORES):
                    for k in range(kw):
                        m.append(((ci, c), k))
        return m

    KC_MAP = kc_map()

    def piece_schedule(pool, gs, tag):
        # returns dict kc -> tile issued just-in-time (with lookahead), plus
        # the per-kc (tile, idx) access list; issue order: t0, then tiers
        done = {}
        order = []
        for kc in range(NKC):
            item, idx = KC_MAP[kc]
            if item not in done:
                done[item] = None
                order.append((kc, item))
        return order

    def pieces_get(state, pool, gs, tag, kc):
        item, idx = KC_MAP[kc]
        if item not in state:
            state[item] = issue_piece(pool, gs, item, tag)
        return state[item][0], idx

    def pieces_prefetch(state, pool, gs, tag, kc, ahead=16):
        # issue pieces for kc..kc+ahead if not yet issued
        for k in range(kc, min(NKC, kc + ahead)):
            item, idx = KC_MAP[k]
            if item not in state:
                state[item] = issue_piece(pool, gs, item, tag)
        # most recently issued piece DMA (runtime-paced via pool WAR)
        return state[KC_MAP[min(NKC - 1, kc + ahead - 1)][0]][1]

    def allgather(locs, gs, which):
        off, n = CH[which]
        out = gs[which]
        if nocc:
            # timing-only variant: local copy in place of the collective
            # (wrong math: fills only the rank-0 slice of the gathered buf)
            nc.sync.dma_start(
                out=out[0:n, :].rearrange("(p kc) n -> p kc n", p=128),
                in_=locs[which][:, :, :])
            return
        nc.gpsimd.collective_compute(
            "AllGather", mybir.AluOpType.bypass, replica_groups=rg,
            ins=[locs[which][:, :, :]], outs=[out[:, :]])

    def store_mt(locs, st, mt):
        for ci, (off, n) in enumerate(CH):
            if off <= mt * 128 < off + n:
                nc.sync.dma_start(
                    out=locs[ci][:, mt - off // 128, :], in_=st[:])
                return

    # after the store of m-tile mt, fire any AG chunk that just completed
    def fire_ags(locs, gs, mt):
        for ci, (off, n) in enumerate(CH):
            if off + n == (mt + 1) * 128:
                allgather(locs, gs, ci)

    # ---- local t GEMM: t = x_loc @ W  ([R, C]), stores + AGs ----
    def t_gemm(lhsT_sb, nkc, w_sb, loc, gs, tag):
        for mt in range(NMT):
            pt = psum.tile([128, 512], F32, tag="acc",
                           name=f"t_{tag}_{mt}_{rep}")
            for kc in range(nkc):
                nc.tensor.matmul(
                    pt[:, 0:C],
                    lhsT_sb[:, kc, mt * 128:(mt + 1) * 128],
                    w_sb[:, kc, :],
                    start=(kc == 0), stop=(kc == nkc - 1))
            st = stg.tile([128, C], BF16, tag="st", name=f"ts_{tag}_{mt}_{rep}")
            nc.vector.tensor_copy(st[:], pt[:, 0:C])
            store_mt(loc, st, mt)
            fire_ags(loc, gs, mt)

    # ---- winv phase: s_loc = f * (winv_rows @ t)  (kc-outer) ----
    def winv_phase(gs, f_sb, sloc, slocgs, tag, fills, pre):
        nmt_mm = 1 if g1mode == "mm1" else NMT
        pstate = {}
        pieces_prefetch(pstate, tpcs, gs, tag, 0, ahead=9)
        accs = [psum.tile([128, 512], F32, tag="acc",
                          name=f"w_{tag}_{mt}_{rep}")
                for mt in range(nmt_mm)]
        wvs = list(pre)
        for kc in range(NKC):
            j = kc // 2
            if g1mode != "wres" and kc % 2 == 0 and j + WPRE < NKC // 2:
                wvs.append(issue_winv_pair(j + WPRE, tag))
            if kc % 4 == 0:
                pieces_prefetch(pstate, tpcs, gs, tag, kc, ahead=6)
            if kc in fills:
                fill_wave(fills[kc], after=wvs[-1][1])
            wv = wvs[j % len(wvs)][0]
            tp, idx = pieces_get(pstate, tpcs, gs, tag, kc)
            for mt in range(nmt_mm):
                nc.tensor.matmul(
                    accs[mt][:, 0:C],
                    wv[:, kc % 2, mt * 128:(mt + 1) * 128],
                    tp[:, idx, :],
                    start=(kc == 0), stop=(kc == NKC - 1))
        for mt in range(nmt_mm):
            st = stg.tile([128, C], BF16, tag="st", name=f"ss_{tag}_{mt}_{rep}")
            nc.scalar.mul(st[:], accs[mt][:, 0:C], f_sb[:, mt:mt + 1])
            if nmt_mm == NMT and slocgs is not None:
                store_mt(sloc, st, mt)
                fire_ags(sloc, slocgs, mt)

    # ---- wave phase: accT = (s_f)^T @ waveT ([C, R] in 4 psums) ----
    def wave_phase(gs, tag, fills, out_cb):
        pstate = {}
        last_piece = pieces_prefetch(pstate, spcs, gs, tag, 0, ahead=9)
        accs = [psum.tile([128, 512], F32, tag="acc",
                          name=f"b_{tag}_{i}_{rep}") for i in range(4)]
        for kc in range(NKC):
            if kc % 4 == 0:
                last_piece = pieces_prefetch(pstate, spcs, gs, tag, kc, ahead=6)
            if kc in fills:
                # chained behind a pool-WAR-paced piece load: fills track G2
                # progress instead of flooding the rings at phase start
                fill_wave(fills[kc], after=last_piece)
            sp, idx = pieces_get(pstate, spcs, gs, tag, kc)
            for mh in range(2):
                for nh in range(2):
                    nc.tensor.matmul(
                        accs[2 * mh + nh][:, :],
                        sp[:, idx, mh * 128:(mh + 1) * 128],
                        waveT_sb[:, kc, nh * 512:(nh + 1) * 512],
                        start=(kc == 0), stop=(kc == NKC - 1))
        out_cb(accs)

    # ================= layer 1 =================
    pre_l1 = [issue_winv_pair(j, "g1") for j in range(WPRE)] if upto >= 2 else []
    if upto >= 2:
        fill_wave(0)
    t_gemm(xT_sb, F // 128, w1_sb, t1loc, t1gs, "t1")
    if upto < 2:
        return
    winv_phase(t1gs, f1_sb, s1loc, s1gs if upto >= 3 else None, "g1",
               {18: 1, 36: 2, 54: 3}, pre_l1)
    if upto < 3:
        return

    def relu_out(accs):
        for mh in range(2):
            for nh in range(2):
                nc.scalar.activation(
                    h1T_sb[:, mh, nh * 512:(nh + 1) * 512],
                    accs[2 * mh + nh][:, :],
                    mybir.ActivationFunctionType.Relu)

    wave_phase(s1gs, "b1",
               {4 * (q - 4): q for q in range(4, 16)}, relu_out)
    if upto < 4:
        return

    # prefetch L2 winv pairs during G2/t2
    pre_l2 = [issue_winv_pair(j, "g3") for j in range(WPRE)] if upto >= 5 else []

    t_gemm(h1T_sb, C // 128, w2_sb, t2loc, t2gs, "t2")
    if upto < 5:
        return

    # ================= layer 2 =================
    winv_phase(t2gs, f2_sb, s2loc, s2gs if upto >= 6 else None, "g3",
               {}, pre_l2)
    if upto < 6:
        return

    def store_out(accs):
        for mh in range(2):
            for nh in range(2):
                ot = ostg.tile([128, 512], F32, tag="ot",
                               name=f"ot_{mh}_{nh}_{rep}")
                nc.vector.tensor_copy(ot[:], accs[2 * mh + nh][:, :])
                nc.sync.dma_start(
                    out=outT.ap()[mh * 128:(mh + 1) * 128,
                                  nh * 512:(nh + 1) * 512],
                    in_=ot[:])

    wave_phase(s2gs, "b2", {}, store_out)


_NC_CACHE = {}


def _get_nc(reps=1, upto=6, nocc=False, g1mode='norm', chunks=None):
    key = (reps, upto, nocc, g1mode, tuple(chunks) if chunks else None)
    if key not in _NC_CACHE:
        _NC_CACHE[key] = build_kernel(reps, upto, nocc, g1mode, chunks)
    return _NC_CACHE[key]


def make_in_maps(input, wavelets, wavelets_inv, W1, W2, filter1, filter2,
                 chunks=None):
    chunks = list(chunks) if chunks is not None else CHUNKS
    input = np.asarray(input, np.float32)
    wavelets = np.asarray(wavelets, np.float32)
    wavelets_inv = np.asarray(wavelets_inv, np.float32)
    W1 = np.asarray(W1, np.float32)
    W2 = np.asarray(W2, np.float32)
    filter1 = np.asarray(filter1, np.float32)
    filter2 = np.asarray(filter2, np.float32)

    pi_parts = []
    for off, n in chunks:
        for c in range(NCORES):
            pi_parts.append(np.arange(c * R + off, c * R + off + n))
    pi = np.concatenate(pi_parts)

    w1b = np.ascontiguousarray(
        W1.reshape(F // 128, 128, C).transpose(1, 0, 2)).astype(NP_BF16)
    w2b = np.ascontiguousarray(
        W2.reshape(C // 128, 128, C).transpose(1, 0, 2)).astype(NP_BF16)
    in_maps = []
    for i in range(NCORES):
        r0, r1 = i * R, (i + 1) * R
        in_maps.append({
            "xT": np.ascontiguousarray(
                input[r0:r1].T.reshape(F // 128, 128, R)
                .transpose(1, 0, 2)).astype(NP_BF16),
            "w1": w1b,
            "w2": w2b,
            "winvT": np.ascontiguousarray(
                wavelets_inv[r0:r1][:, pi].T.reshape(NKC // 2, 2, 128, R)
                .transpose(0, 2, 1, 3)).astype(NP_BF16),
            "waveT": np.ascontiguousarray(
                wavelets[r0:r1][:, pi].T.reshape(NKC // 4, 4, 128, R)
                .transpose(0, 2, 1, 3)).astype(NP_BF16),
            "f1": np.ascontiguousarray(filter1[r0:r1].reshape(NMT, 128).T),
            "f2": np.ascontiguousarray(filter2[r0:r1].reshape(NMT, 128).T),
        })
    return in_maps


def run(in_maps, trace=False, **kw):
    nc = _get_nc()
    return bass_utils.run_bass_kernel_spmd(
        nc, in_maps, core_ids=list(range(NCORES)), trace=trace, **kw)


def kernel(input, wavelets, wavelets_inv, W1, W2, filter1, filter2):
    in_maps = make_in_maps(
        input, wavelets, wavelets_inv, W1, W2, filter1, filter2)
    res = run(in_maps)
    out = np.empty((N, C), np.float32)
    for i in range(NCORES):
        out[i * R:(i + 1) * R, :] = res.results[i]["outT"].T
    return out
